# revision 1
# baseline (speedup 1.0000x reference)
"""Trainium2 Bass kernel for nn_CMIP_75883482186148 (histogram_binning).

The reference computes per-channel thresholds from tiny [C] weight vectors
(histogram valley search), derives channel masks m1/m2, and then does
    y1 = where(m1[None,:,None,None], x0, x1)
    y2 = where(m2[None,:,None,None], x1, x0)
over [B=8, C=256, H=128, W=128] f32 tensors.

Every output channel is a verbatim copy of one input's channel slab, so the
device work is pure data movement.  Strategy:
  - threshold/mask search is O(C) scalar work -> done on host with a
    bit-exact float32 port of the jax reference (kernel launch params).
  - shard batch across the 8 NeuronCores (1 batch element each, SPMD).
  - on each core, emit one DRAM->DRAM HWDGE DMA per run of consecutive
    channels that share a source (typically only ~10-30 runs total since the
    masks are heavily skewed), split across the SP and ACT HWDGE rings.
This keeps the kernel at the HBM roofline (read 32 MiB + write 32 MiB per
core) with no compute-engine involvement.
"""

import numpy as np

B, C, H, W = 8, 256, 128, 128
F = H * W  # contiguous f32 elements per (batch, channel) slab
N_CORES = 8

_PROGRAM_CACHE: dict = {}
LAST_RESULTS = None  # stash of BassKernelResults for test harness introspection


def _mask(w: np.ndarray) -> np.ndarray:
    """Bit-exact float32 port of reference.search_threshold + (|w| >= t)."""
    b = np.abs(np.asarray(w, dtype=np.float32))
    bins = b.shape[0]
    wmin = b.min()
    wmax = b.max()
    idx = np.clip(
        np.floor((b - wmin) / (wmax - wmin) * np.float32(bins)).astype(np.int32),
        0,
        bins - 1,
    )
    hist = np.zeros(bins, dtype=np.float32)
    np.add.at(hist, idx, np.float32(1))
    d = np.diff(hist)
    cond = (d[:-1] <= 0) & (d[1:] > 0)
    i = np.int32(np.argmax(cond)) if cond.any() else np.int32(0)
    t = wmin + np.float32(i + 2) * (wmax - wmin) / np.float32(bins)
    return b >= t


def _runs(mask: np.ndarray):
    """Maximal runs of equal mask value: list of (start, end, value)."""
    out = []
    s = 0
    n = len(mask)
    for c in range(1, n + 1):
        if c == n or bool(mask[c]) != bool(mask[s]):
            out.append((s, c, bool(mask[s])))
            s = c
    return out


def _build_program(m1: np.ndarray, m2: np.ndarray):
    import concourse.bass as bass
    import concourse.mybir as mybir

    f32 = mybir.dt.float32
    nc = bass.Bass(trn_type="TRN2")
    x0 = nc.dram_tensor("x0", [C, F], f32, kind="ExternalInput")
    x1 = nc.dram_tensor("x1", [C, F], f32, kind="ExternalInput")
    y1 = nc.dram_tensor("y1", [C, F], f32, kind="ExternalOutput")
    y2 = nc.dram_tensor("y2", [C, F], f32, kind="ExternalOutput")

    runs1 = _runs(m1)
    runs2 = _runs(m2)

    with (
        nc.semaphore("dma1") as s1,
        nc.semaphore("dma2") as s2,
        nc.Block() as block,
    ):

        @block.sync
        def _(sync):
            n = 0
            for a, b, v in runs1:
                src = x0 if v else x1
                sync.dma_start(y1[a:b, :], src[a:b, :]).then_inc(s1, 16)
                n += 16
            sync.wait_ge(s1, n)

        @block.scalar
        def _(scalar):
            n = 0
            for a, b, v in runs2:
                src = x1 if v else x0
                scalar.dma_start(y2[a:b, :], src[a:b, :]).then_inc(s2, 16)
                n += 16
            scalar.wait_ge(s2, n)

    return nc


def kernel(x0, x1, w1, w2):
    global LAST_RESULTS
    from concourse.bass_utils import run_bass_kernel_spmd

    x0 = np.ascontiguousarray(np.asarray(x0, dtype=np.float32))
    x1 = np.ascontiguousarray(np.asarray(x1, dtype=np.float32))
    assert x0.shape == (B, C, H, W) and x1.shape == (B, C, H, W)

    m1 = _mask(w1)
    m2 = _mask(w2)
    key = (m1.tobytes(), m2.tobytes())
    nc = _PROGRAM_CACHE.get(key)
    if nc is None:
        nc = _build_program(m1, m2)
        _PROGRAM_CACHE[key] = nc

    in_maps = [
        {"x0": x0[b].reshape(C, F), "x1": x1[b].reshape(C, F)} for b in range(B)
    ]
    res = run_bass_kernel_spmd(nc, in_maps, core_ids=list(range(N_CORES)))
    LAST_RESULTS = res

    y1 = np.stack([res.results[b]["y1"] for b in range(B)]).reshape(B, C, H, W)
    y2 = np.stack([res.results[b]["y2"] for b in range(B)]).reshape(B, C, H, W)
    return (y1, y2)


# revision 2
# speedup vs baseline: 132346.6162x; 132346.6162x over previous
"""Trainium2 Bass kernel for nn_CMIP_75883482186148 (histogram_binning).

The reference computes per-channel thresholds from tiny [C] weight vectors
(histogram valley search), derives channel masks m1/m2, and then does
    y1 = where(m1[None,:,None,None], x0, x1)
    y2 = where(m2[None,:,None,None], x1, x0)
over [B=8, C=256, H=128, W=128] f32 tensors.

Every output channel is a verbatim copy of one input's channel slab, so the
device work is pure data movement.  Strategy:
  - threshold/mask search is O(C) scalar work -> done on host with a
    bit-exact float32 port of the jax reference (kernel launch params).
  - shard batch across the 8 NeuronCores (1 batch element each, SPMD).
  - on each core, emit one DRAM->DRAM HWDGE DMA per run of consecutive
    channels that share a source (typically only ~10-30 runs total since the
    masks are heavily skewed), split across the SP and ACT HWDGE rings.
This keeps the kernel at the HBM roofline (read 32 MiB + write 32 MiB per
core) with no compute-engine involvement.
"""

import numpy as np

B, C, H, W = 8, 256, 128, 128
F = H * W  # contiguous f32 elements per (batch, channel) slab
N_CORES = 8

_PROGRAM_CACHE: dict = {}
LAST_RESULTS = None  # stash of BassKernelResults for test harness introspection


def _mask(w: np.ndarray) -> np.ndarray:
    """Bit-exact float32 port of reference.search_threshold + (|w| >= t)."""
    b = np.abs(np.asarray(w, dtype=np.float32))
    bins = b.shape[0]
    wmin = b.min()
    wmax = b.max()
    idx = np.clip(
        np.floor((b - wmin) / (wmax - wmin) * np.float32(bins)).astype(np.int32),
        0,
        bins - 1,
    )
    hist = np.zeros(bins, dtype=np.float32)
    np.add.at(hist, idx, np.float32(1))
    d = np.diff(hist)
    cond = (d[:-1] <= 0) & (d[1:] > 0)
    i = np.int32(np.argmax(cond)) if cond.any() else np.int32(0)
    t = wmin + np.float32(i + 2) * (wmax - wmin) / np.float32(bins)
    return b >= t


def _runs(mask: np.ndarray):
    """Maximal runs of equal mask value: list of (start, end, value)."""
    out = []
    s = 0
    n = len(mask)
    for c in range(1, n + 1):
        if c == n or bool(mask[c]) != bool(mask[s]):
            out.append((s, c, bool(mask[s])))
            s = c
    return out


def _build_program(m1: np.ndarray, m2: np.ndarray):
    import concourse.bass as bass
    import concourse.mybir as mybir

    f32 = mybir.dt.float32
    nc = bass.Bass(trn_type="TRN2")
    x0 = nc.dram_tensor("x0", [C, F], f32, kind="ExternalInput")
    x1 = nc.dram_tensor("x1", [C, F], f32, kind="ExternalInput")
    y1 = nc.dram_tensor("y1", [C, F], f32, kind="ExternalOutput")
    y2 = nc.dram_tensor("y2", [C, F], f32, kind="ExternalOutput")

    runs1 = _runs(m1)
    runs2 = _runs(m2)

    with (
        nc.semaphore("dma1") as s1,
        nc.semaphore("dma2") as s2,
        nc.Block() as block,
    ):

        @block.sync
        def _(sync):
            n = 0
            for a, b, v in runs1:
                src = x0 if v else x1
                sync.dma_start(y1[a:b, :], src[a:b, :]).then_inc(s1, 16)
                n += 16
            sync.wait_ge(s1, n)

        @block.scalar
        def _(scalar):
            n = 0
            for a, b, v in runs2:
                src = x1 if v else x0
                scalar.dma_start(y2[a:b, :], src[a:b, :]).then_inc(s2, 16)
                n += 16
            scalar.wait_ge(s2, n)

    return nc


def prepare(x0, x1, w1, w2):
    """Build (cached) bass program + per-core input maps for these inputs."""
    x0 = np.ascontiguousarray(np.asarray(x0, dtype=np.float32))
    x1 = np.ascontiguousarray(np.asarray(x1, dtype=np.float32))
    assert x0.shape == (B, C, H, W) and x1.shape == (B, C, H, W)

    m1 = _mask(w1)
    m2 = _mask(w2)
    key = (m1.tobytes(), m2.tobytes())
    nc = _PROGRAM_CACHE.get(key)
    if nc is None:
        nc = _build_program(m1, m2)
        _PROGRAM_CACHE[key] = nc

    in_maps = [
        {"x0": x0[b].reshape(C, F), "x1": x1[b].reshape(C, F)} for b in range(B)
    ]
    return nc, in_maps


def kernel(x0, x1, w1, w2):
    global LAST_RESULTS
    from concourse.bass_utils import run_bass_kernel_spmd

    nc, in_maps = prepare(x0, x1, w1, w2)
    res = run_bass_kernel_spmd(nc, in_maps, core_ids=list(range(N_CORES)))
    LAST_RESULTS = res

    y1 = np.stack([res.results[b]["y1"] for b in range(B)]).reshape(B, C, H, W)
    y2 = np.stack([res.results[b]["y2"] for b in range(B)]).reshape(B, C, H, W)
    return (y1, y2)


# revision 3
# speedup vs baseline: 849309.9773x; 6.4173x over previous
"""Trainium2 Bass kernel for nn_CMIP_75883482186148 (histogram_binning).

Reference semantics: thresholds t1/t2 are found by a histogram-valley search
over |w1|/|w2| (C=256 channels); channel masks m1 = |w1|>=t1, m2 = |w2|>=t2;
then over [B=8, C=256, H=128, W=128] f32 tensors:
    y1 = where(m1[None,:,None,None], x0, x1)
    y2 = where(m2[None,:,None,None], x1, x0)

Every output channel is a verbatim copy of one input's channel slab, so the
device work is pure data movement.  Strategy:

  * The O(C) threshold search is bit-exactly ported to host float32 numpy and
    computed as kernel launch parameters (it decides the DMA pattern).
  * Batch is sharded across the 8 NeuronCores (1 batch element each, SPMD).
  * Primary path (in-place): y1 aliases x0's device buffer and y2 aliases
    x1's (XLA input-output aliasing through the NKI custom-BIR-kernel
    lowering, inputs donated).  The NEFF then only has to patch the channels
    where the output differs from the aliased input (y1 takes x1 on ~m1
    channels, y2 takes x0 on ~m2 channels) with DRAM->DRAM HWDGE DMAs.
    Channels in S = ~m1 & ~m2 swap between the two buffers, so those are
    staged through an internal DRAM scratch first.  Moved bytes:
    (|~m1| + |~m2| + 2|S|) * 64 KiB per core, <= and typically far below the
    64 MiB/core a full rewrite costs (the masks are heavily skewed: ~97-99%
    of channels keep the aliased data).
  * Fallback path (full copy): one DRAM->DRAM DMA per run of consecutive
    channels sharing a source, split over the SP/ACT HWDGE rings; runs at
    the HBM roofline (~115-140 us/core).
"""

import numpy as np

B, C, H, W = 8, 256, 128, 128
F = H * W  # contiguous f32 elements per (batch, channel) slab
N_CORES = 8

INPLACE = True  # primary path; set False to force the full-copy kernel

_INPLACE_CACHE: dict = {}
_COPY_CACHE: dict = {}
LAST_RESULTS = None  # BassKernelResults stash (fallback path only)


def _mask(w: np.ndarray) -> np.ndarray:
    """Bit-exact float32 port of reference.search_threshold + (|w| >= t)."""
    b = np.abs(np.asarray(w, dtype=np.float32))
    bins = b.shape[0]
    wmin = b.min()
    wmax = b.max()
    idx = np.clip(
        np.floor((b - wmin) / (wmax - wmin) * np.float32(bins)).astype(np.int32),
        0,
        bins - 1,
    )
    hist = np.zeros(bins, dtype=np.float32)
    np.add.at(hist, idx, np.float32(1))
    d = np.diff(hist)
    cond = (d[:-1] <= 0) & (d[1:] > 0)
    i = np.int32(np.argmax(cond)) if cond.any() else np.int32(0)
    t = wmin + np.float32(i + 2) * (wmax - wmin) / np.float32(bins)
    return b >= t


def _runs(mask: np.ndarray, value: bool | None = None):
    """Maximal runs of equal mask value: [(start, end, value)].
    If `value` given, only runs with that value, as [(start, end)]."""
    out = []
    s = 0
    n = len(mask)
    for c in range(1, n + 1):
        if c == n or bool(mask[c]) != bool(mask[s]):
            out.append((s, c, bool(mask[s])))
            s = c
    if value is None:
        return out
    return [(a, b) for a, b, v in out if v == value]


# ---------------------------------------------------------------- in-place --


def _build_patch_program(m1: np.ndarray, m2: np.ndarray):
    """Patch-only program: y1/y2 are bound to x0/x1's buffers by XLA
    aliasing; only differing channels are written.  S-channels (both masks
    False) swap data between the buffers, so they stage via DRAM scratch."""
    import concourse.bass as bass
    import concourse.mybir as mybir

    f32 = mybir.dt.float32
    nc = bass.Bass(trn_type="TRN2", target_bir_lowering=True,
                   enable_partition_id=False)
    x0 = nc.dram_tensor("x0", [C, F], f32, kind="ExternalInput")
    x1 = nc.dram_tensor("x1", [C, F], f32, kind="ExternalInput")
    y1 = nc.dram_tensor("y1", [C, F], f32, kind="ExternalOutput")
    y2 = nc.dram_tensor("y2", [C, F], f32, kind="ExternalOutput")

    s_mask = (~m1) & (~m2)  # swap channels: y1[c]<-x1[c] AND y2[c]<-x0[c]
    s_runs = _runs(s_mask, True)
    s_total = int(s_mask.sum())
    # direct patches: source channel is never overwritten by the other side
    p1_runs = _runs((~m1) & m2, True)  # y1[c] <- x1[c], x1[c] stays intact
    p2_runs = _runs((~m2) & m1, True)  # y2[c] <- x0[c], x0[c] stays intact

    scr0 = scr1 = None
    if s_total:
        scr0 = nc.dram_tensor("scr0", [s_total, F], f32, kind="Internal")
        scr1 = nc.dram_tensor("scr1", [s_total, F], f32, kind="Internal")

    with nc.semaphore("dma1") as s1, nc.Block() as block:

        @block.sync
        def _(sync):
            n = 0
            # stage the swap set first (reads of both buffers)
            o = 0
            for a, b in s_runs:
                k = b - a
                sync.dma_start(scr0[o : o + k, :], x0[a:b, :]).then_inc(s1, 16)
                sync.dma_start(scr1[o : o + k, :], x1[a:b, :]).then_inc(s1, 16)
                n += 32
                o += k
            n_stage = n
            # direct patches can go while staging drains
            for a, b in p1_runs:
                sync.dma_start(y1[a:b, :], x1[a:b, :]).then_inc(s1, 16)
                n += 16
            for a, b in p2_runs:
                sync.dma_start(y2[a:b, :], x0[a:b, :]).then_inc(s1, 16)
                n += 16
            if s_total:
                # swap-set writes must wait for the staged reads
                sync.wait_ge(s1, n_stage)
                o = 0
                for a, b in s_runs:
                    k = b - a
                    sync.dma_start(y1[a:b, :], scr1[o : o + k, :]).then_inc(s1, 16)
                    sync.dma_start(y2[a:b, :], scr0[o : o + k, :]).then_inc(s1, 16)
                    n += 32
                    o += k
            if n:
                sync.wait_ge(s1, n)

    return nc


def _get_inplace_fn(key, m1, m2):
    cached = _INPLACE_CACHE.get(key)
    if cached is not None:
        return cached

    import jax
    from jax.experimental.shard_map import shard_map
    from jax.sharding import Mesh, PartitionSpec as P

    from concourse.bass2jax import _bass_exec_p, install_neuronx_cc_hook

    install_neuronx_cc_hook()
    nc = _build_patch_program(m1, m2)
    aval = jax.core.ShapedArray((C, F), np.float32)

    def _body(a0, a1):
        outs = _bass_exec_p.bind(
            a0,
            a1,
            out_avals=(aval, aval),
            in_names=("x0", "x1"),
            out_names=("y1", "y2"),
            lowering_input_output_aliases=((0, 0), (1, 1)),
            sim_require_finite=True,
            sim_require_nnan=True,
            nc=nc,
        )
        return tuple(outs)

    devices = jax.devices()[:N_CORES]
    assert len(devices) == N_CORES, f"need {N_CORES} cores, got {len(devices)}"
    mesh = Mesh(np.asarray(devices), ("core",))
    fn = jax.jit(
        shard_map(
            _body,
            mesh=mesh,
            in_specs=(P("core"), P("core")),
            out_specs=(P("core"), P("core")),
            check_rep=False,
        ),
        donate_argnums=(0, 1),
    )
    _INPLACE_CACHE[key] = fn
    return fn


# --------------------------------------------------------------- full copy --


def _build_copy_program(m1: np.ndarray, m2: np.ndarray):
    """Full rewrite of y1/y2: one DRAM->DRAM DMA per same-source channel
    run, y1-runs on the SP HWDGE ring and y2-runs on the ACT ring."""
    import concourse.bass as bass
    import concourse.mybir as mybir

    f32 = mybir.dt.float32
    nc = bass.Bass(trn_type="TRN2")
    x0 = nc.dram_tensor("x0", [C, F], f32, kind="ExternalInput")
    x1 = nc.dram_tensor("x1", [C, F], f32, kind="ExternalInput")
    y1 = nc.dram_tensor("y1", [C, F], f32, kind="ExternalOutput")
    y2 = nc.dram_tensor("y2", [C, F], f32, kind="ExternalOutput")

    with (
        nc.semaphore("dma1") as s1,
        nc.semaphore("dma2") as s2,
        nc.Block() as block,
    ):

        @block.sync
        def _(sync):
            n = 0
            for a, b, v in _runs(m1):
                src = x0 if v else x1
                sync.dma_start(y1[a:b, :], src[a:b, :]).then_inc(s1, 16)
                n += 16
            sync.wait_ge(s1, n)

        @block.scalar
        def _(scalar):
            n = 0
            for a, b, v in _runs(m2):
                src = x1 if v else x0
                scalar.dma_start(y2[a:b, :], src[a:b, :]).then_inc(s2, 16)
                n += 16
            scalar.wait_ge(s2, n)

    return nc


def prepare(x0, x1, w1, w2):
    """(fallback path) build cached full-copy program + per-core in_maps."""
    x0 = np.ascontiguousarray(np.asarray(x0, dtype=np.float32))
    x1 = np.ascontiguousarray(np.asarray(x1, dtype=np.float32))
    m1 = _mask(w1)
    m2 = _mask(w2)
    key = (m1.tobytes(), m2.tobytes())
    nc = _COPY_CACHE.get(key)
    if nc is None:
        nc = _build_copy_program(m1, m2)
        _COPY_CACHE[key] = nc
    in_maps = [
        {"x0": x0[b].reshape(C, F), "x1": x1[b].reshape(C, F)} for b in range(B)
    ]
    return nc, in_maps


# ------------------------------------------------------------------ kernel --


def kernel(x0, x1, w1, w2):
    global LAST_RESULTS
    x0 = np.ascontiguousarray(np.asarray(x0, dtype=np.float32))
    x1 = np.ascontiguousarray(np.asarray(x1, dtype=np.float32))
    assert x0.shape == (B, C, H, W) and x1.shape == (B, C, H, W)

    if INPLACE:
        m1 = _mask(w1)
        m2 = _mask(w2)
        key = (m1.tobytes(), m2.tobytes())
        fn = _get_inplace_fn(key, m1, m2)
        o1, o2 = fn(x0.reshape(B * C, F), x1.reshape(B * C, F))
        y1 = np.asarray(o1).reshape(B, C, H, W)
        y2 = np.asarray(o2).reshape(B, C, H, W)
        return (y1, y2)

    from concourse.bass_utils import run_bass_kernel_spmd

    nc, in_maps = prepare(x0, x1, w1, w2)
    res = run_bass_kernel_spmd(nc, in_maps, core_ids=list(range(N_CORES)))
    LAST_RESULTS = res
    y1 = np.stack([res.results[b]["y1"] for b in range(B)]).reshape(B, C, H, W)
    y2 = np.stack([res.results[b]["y2"] for b in range(B)]).reshape(B, C, H, W)
    return (y1, y2)


# revision 4
# speedup vs baseline: 885962.3336x; 1.0432x over previous
"""Trainium2 Bass kernel for nn_CMIP_75883482186148 (histogram_binning).

Reference semantics: thresholds t1/t2 are found by a histogram-valley search
over |w1|/|w2| (C=256 channels); channel masks m1 = |w1|>=t1, m2 = |w2|>=t2;
then over [B=8, C=256, H=128, W=128] f32 tensors:
    y1 = where(m1[None,:,None,None], x0, x1)
    y2 = where(m2[None,:,None,None], x1, x0)

Every output channel is a verbatim copy of one input's channel slab, so the
device work is pure data movement.  Strategy:

  * The O(C) threshold search is bit-exactly ported to host float32 numpy and
    computed as kernel launch parameters (it decides the DMA pattern).
  * Batch is sharded across the 8 NeuronCores (1 batch element each, SPMD).
  * Primary path (in-place): y1 aliases x0's device buffer and y2 aliases
    x1's (XLA input-output aliasing through the NKI custom-BIR-kernel
    lowering, inputs donated).  The NEFF then only has to patch the channels
    where the output differs from the aliased input (y1 takes x1 on ~m1
    channels, y2 takes x0 on ~m2 channels) with DRAM->DRAM HWDGE DMAs.
    Channels in S = ~m1 & ~m2 swap between the two buffers, so those are
    staged through an internal DRAM scratch first.  Moved bytes:
    (|~m1| + |~m2| + 2|S|) * 64 KiB per core, <= and typically far below the
    64 MiB/core a full rewrite costs (the masks are heavily skewed: ~97-99%
    of channels keep the aliased data).
  * Fallback path (full copy): one DRAM->DRAM DMA per run of consecutive
    channels sharing a source, split over the SP/ACT HWDGE rings; runs at
    the HBM roofline (~115-140 us/core).
"""

import numpy as np

B, C, H, W = 8, 256, 128, 128
F = H * W  # contiguous f32 elements per (batch, channel) slab
N_CORES = 8

INPLACE = True  # primary path; set False to force the full-copy kernel

_INPLACE_CACHE: dict = {}
_COPY_CACHE: dict = {}
LAST_RESULTS = None  # BassKernelResults stash (fallback path only)


def _mask(w: np.ndarray) -> np.ndarray:
    """Bit-exact float32 port of reference.search_threshold + (|w| >= t)."""
    b = np.abs(np.asarray(w, dtype=np.float32))
    bins = b.shape[0]
    wmin = b.min()
    wmax = b.max()
    idx = np.clip(
        np.floor((b - wmin) / (wmax - wmin) * np.float32(bins)).astype(np.int32),
        0,
        bins - 1,
    )
    hist = np.zeros(bins, dtype=np.float32)
    np.add.at(hist, idx, np.float32(1))
    d = np.diff(hist)
    cond = (d[:-1] <= 0) & (d[1:] > 0)
    i = np.int32(np.argmax(cond)) if cond.any() else np.int32(0)
    t = wmin + np.float32(i + 2) * (wmax - wmin) / np.float32(bins)
    return b >= t


def _runs(mask: np.ndarray, value: bool | None = None):
    """Maximal runs of equal mask value: [(start, end, value)].
    If `value` given, only runs with that value, as [(start, end)]."""
    out = []
    s = 0
    n = len(mask)
    for c in range(1, n + 1):
        if c == n or bool(mask[c]) != bool(mask[s]):
            out.append((s, c, bool(mask[s])))
            s = c
    if value is None:
        return out
    return [(a, b) for a, b, v in out if v == value]


# ---------------------------------------------------------------- in-place --


def _build_patch_program(m1: np.ndarray, m2: np.ndarray):
    """Patch-only program: y1/y2 are bound to x0/x1's buffers by XLA
    aliasing; only differing channels are written.  S-channels (both masks
    False) swap data between the buffers, so they stage via DRAM scratch."""
    import concourse.bass as bass
    import concourse.mybir as mybir

    f32 = mybir.dt.float32
    nc = bass.Bass(trn_type="TRN2", target_bir_lowering=True,
                   enable_partition_id=False)
    x0 = nc.dram_tensor("x0", [C, F], f32, kind="ExternalInput")
    x1 = nc.dram_tensor("x1", [C, F], f32, kind="ExternalInput")
    y1 = nc.dram_tensor("y1", [C, F], f32, kind="ExternalOutput")
    y2 = nc.dram_tensor("y2", [C, F], f32, kind="ExternalOutput")

    s_mask = (~m1) & (~m2)  # swap channels: y1[c]<-x1[c] AND y2[c]<-x0[c]
    s_runs = _runs(s_mask, True)
    s_total = int(s_mask.sum())
    # direct patches: source channel is never overwritten by the other side
    p1_runs = _runs((~m1) & m2, True)  # y1[c] <- x1[c], x1[c] stays intact
    p2_runs = _runs((~m2) & m1, True)  # y2[c] <- x0[c], x0[c] stays intact

    scr0 = scr1 = None
    if s_total:
        scr0 = nc.dram_tensor("scr0", [s_total, F], f32, kind="Internal")
        scr1 = nc.dram_tensor("scr1", [s_total, F], f32, kind="Internal")

    # interleave direct patches across both HWDGE rings (SP + ACT) to halve
    # the per-instruction issue serialization on the sequencers
    direct = [(y1, x1, a, b) for a, b in p1_runs] + [(y2, x0, a, b) for a, b in p2_runs]
    direct_sp = direct[0::2]
    direct_act = direct[1::2]

    with (
        nc.semaphore("dma1") as s1,
        nc.semaphore("dma2") as s2,
        nc.Block() as block,
    ):

        @block.sync
        def _(sync):
            n = 0
            # stage the swap set first (reads of both buffers)
            o = 0
            for a, b in s_runs:
                k = b - a
                sync.dma_start(scr0[o : o + k, :], x0[a:b, :]).then_inc(s1, 16)
                sync.dma_start(scr1[o : o + k, :], x1[a:b, :]).then_inc(s1, 16)
                n += 32
                o += k
            n_stage = n
            # direct patches can go while staging drains
            for dst, src, a, b in direct_sp:
                sync.dma_start(dst[a:b, :], src[a:b, :]).then_inc(s1, 16)
                n += 16
            if s_total:
                # swap-set writes must wait for the staged reads
                sync.wait_ge(s1, n_stage)
                o = 0
                for a, b in s_runs:
                    k = b - a
                    sync.dma_start(y1[a:b, :], scr1[o : o + k, :]).then_inc(s1, 16)
                    sync.dma_start(y2[a:b, :], scr0[o : o + k, :]).then_inc(s1, 16)
                    n += 32
                    o += k
            if n:
                sync.wait_ge(s1, n)

        @block.scalar
        def _(scalar):
            n = 0
            for dst, src, a, b in direct_act:
                scalar.dma_start(dst[a:b, :], src[a:b, :]).then_inc(s2, 16)
                n += 16
            if n:
                scalar.wait_ge(s2, n)

    return nc


def _get_inplace_fn(key, m1, m2):
    cached = _INPLACE_CACHE.get(key)
    if cached is not None:
        return cached

    import jax
    from jax.experimental.shard_map import shard_map
    from jax.sharding import Mesh, PartitionSpec as P

    from concourse.bass2jax import _bass_exec_p, install_neuronx_cc_hook

    install_neuronx_cc_hook()
    nc = _build_patch_program(m1, m2)
    aval = jax.core.ShapedArray((C, F), np.float32)

    def _body(a0, a1):
        outs = _bass_exec_p.bind(
            a0,
            a1,
            out_avals=(aval, aval),
            in_names=("x0", "x1"),
            out_names=("y1", "y2"),
            lowering_input_output_aliases=((0, 0), (1, 1)),
            sim_require_finite=True,
            sim_require_nnan=True,
            nc=nc,
        )
        return tuple(outs)

    devices = jax.devices()[:N_CORES]
    assert len(devices) == N_CORES, f"need {N_CORES} cores, got {len(devices)}"
    mesh = Mesh(np.asarray(devices), ("core",))
    fn = jax.jit(
        shard_map(
            _body,
            mesh=mesh,
            in_specs=(P("core"), P("core")),
            out_specs=(P("core"), P("core")),
            check_rep=False,
        ),
        donate_argnums=(0, 1),
    )
    _INPLACE_CACHE[key] = fn
    return fn


# --------------------------------------------------------------- full copy --


def _build_copy_program(m1: np.ndarray, m2: np.ndarray):
    """Full rewrite of y1/y2: one DRAM->DRAM DMA per same-source channel
    run, y1-runs on the SP HWDGE ring and y2-runs on the ACT ring."""
    import concourse.bass as bass
    import concourse.mybir as mybir

    f32 = mybir.dt.float32
    nc = bass.Bass(trn_type="TRN2")
    x0 = nc.dram_tensor("x0", [C, F], f32, kind="ExternalInput")
    x1 = nc.dram_tensor("x1", [C, F], f32, kind="ExternalInput")
    y1 = nc.dram_tensor("y1", [C, F], f32, kind="ExternalOutput")
    y2 = nc.dram_tensor("y2", [C, F], f32, kind="ExternalOutput")

    with (
        nc.semaphore("dma1") as s1,
        nc.semaphore("dma2") as s2,
        nc.Block() as block,
    ):

        @block.sync
        def _(sync):
            n = 0
            for a, b, v in _runs(m1):
                src = x0 if v else x1
                sync.dma_start(y1[a:b, :], src[a:b, :]).then_inc(s1, 16)
                n += 16
            sync.wait_ge(s1, n)

        @block.scalar
        def _(scalar):
            n = 0
            for a, b, v in _runs(m2):
                src = x1 if v else x0
                scalar.dma_start(y2[a:b, :], src[a:b, :]).then_inc(s2, 16)
                n += 16
            scalar.wait_ge(s2, n)

    return nc


def prepare(x0, x1, w1, w2):
    """(fallback path) build cached full-copy program + per-core in_maps."""
    x0 = np.ascontiguousarray(np.asarray(x0, dtype=np.float32))
    x1 = np.ascontiguousarray(np.asarray(x1, dtype=np.float32))
    m1 = _mask(w1)
    m2 = _mask(w2)
    key = (m1.tobytes(), m2.tobytes())
    nc = _COPY_CACHE.get(key)
    if nc is None:
        nc = _build_copy_program(m1, m2)
        _COPY_CACHE[key] = nc
    in_maps = [
        {"x0": x0[b].reshape(C, F), "x1": x1[b].reshape(C, F)} for b in range(B)
    ]
    return nc, in_maps


# ------------------------------------------------------------------ kernel --


def kernel(x0, x1, w1, w2):
    global LAST_RESULTS
    x0 = np.ascontiguousarray(np.asarray(x0, dtype=np.float32))
    x1 = np.ascontiguousarray(np.asarray(x1, dtype=np.float32))
    assert x0.shape == (B, C, H, W) and x1.shape == (B, C, H, W)

    if INPLACE:
        m1 = _mask(w1)
        m2 = _mask(w2)
        key = (m1.tobytes(), m2.tobytes())
        fn = _get_inplace_fn(key, m1, m2)
        o1, o2 = fn(x0.reshape(B * C, F), x1.reshape(B * C, F))
        y1 = np.asarray(o1).reshape(B, C, H, W)
        y2 = np.asarray(o2).reshape(B, C, H, W)
        return (y1, y2)

    from concourse.bass_utils import run_bass_kernel_spmd

    nc, in_maps = prepare(x0, x1, w1, w2)
    res = run_bass_kernel_spmd(nc, in_maps, core_ids=list(range(N_CORES)))
    LAST_RESULTS = res
    y1 = np.stack([res.results[b]["y1"] for b in range(B)]).reshape(B, C, H, W)
    y2 = np.stack([res.results[b]["y2"] for b in range(B)]).reshape(B, C, H, W)
    return (y1, y2)
